# revision 1
# baseline (speedup 1.0000x reference)
"""Trainium2 Bass kernel for DPLossV2 soft-rank MSE loss.

Computes, for x:[512,512], z:[512,64]:
    dist_x = cdist(x), dist_z = cdist(z)           (pairwise Euclidean)
    rank_m[i,j] = 1 + sum_k sigmoid((m[i,k]-m[i,j])/tau)
    loss = mean((rank_z - rank_x)**2)
returns (loss, loss, 0.0) since lambda_rank=1, lambda_pairdist=0.

Sharding: the 512 rows of both distance matrices split across 8
NeuronCores (64 rows each). Per core, the x-row slab occupies SBUF
partitions 0-63 and the z-row slab partitions 64-127, so every ScalarE
instruction processes both matrices at once (full 128-lane utilization).

The O(n^3) soft-rank is done triangularly: instruction k evaluates
    T_k[p, j] = sigmoid(S[p,k] - S[p,j])    for j < k only
(ACT with per-partition bias S[:,k], scale=-1, PSUM-sourced input),
which halves the sigmoid payload; the j > k half follows from
sigmoid(u) + sigmoid(-u) = 1:
    rank[p,j] = 1.5 + j + sum_{k>j} T_k[p,j] - sum_{k<j} T_j[p,k]
PE accumulates the first (cross-instruction) sum into PSUM via
identity-matmuls; VectorE row-reduces each T_k into W[:,k] for the
second. The device outputs V = R_acc - W per core; the host forms
D = V[z-half] - V[x-half] (the 1.5+j terms cancel) and the scalar MSE
partial sums are reduced across the 8 cores in float64.

Hardware-measured: ~287 us on trn2 (ScalarE-bound: 511 sigmoid ACTs at
~300ns fixed + ~1ns/elem; this is the structural floor for the
per-k-bias formulation since a core's 128 partitions x 512 instruction
slots exactly cover its 1024 row-matrix units x 512 columns).
"""

import numpy as np
from contextlib import ExitStack

import concourse.bass as bass
import concourse.bacc as bacc
import concourse.mybir as mybir
import concourse.tile as tile
from concourse.bass_utils import run_bass_kernel_spmd

N = 512        # number of rows / rank dimension
DX = 512       # x feature dim
DZ = 64        # z feature dim
NCORES = 8
ROWS = N // NCORES          # 64 rows per core
F32 = mybir.dt.float32
BF16 = mybir.dt.bfloat16
AF = mybir.ActivationFunctionType
TAU = 1.0


def _build() -> bass.Bass:
    nc = bacc.Bacc()

    # Per-core inputs. Each matmul must depend on a single DMA (the LDW
    # sync-wait slot is limited), so rhs|lhsT are concatenated per tensor:
    # columns 0..N-1 = full transposed matrix (rhs), N..N+ROWS-1 = this
    # core's slab columns (lhsT). The two aux contraction rows fold the
    # squared norms into the matmul: G' = x_i.x_j - sq_i/2 - sq_j/2.
    W = N + ROWS
    xcat = nc.dram_tensor("xcat", [DX, W], F32, kind="ExternalInput")
    zcat = nc.dram_tensor("zcat", [DZ, W], F32, kind="ExternalInput")
    acx = nc.dram_tensor("acx", [2, W], F32, kind="ExternalInput")
    acz = nc.dram_tensor("acz", [2, W], F32, kind="ExternalInput")
    ident = nc.dram_tensor("ident", [128, 128], F32, kind="ExternalInput")
    rout = nc.dram_tensor("rout", [128, N], F32, kind="ExternalOutput")

    nb = DX // 128  # xcat partition blocks

    with tile.TileContext(nc) as tc:
        with ExitStack() as ctx:
            cp = ctx.enter_context(tc.tile_pool(name="const", bufs=1))
            tkp = ctx.enter_context(tc.tile_pool(name="tk", bufs=16))
            pp = ctx.enter_context(tc.tile_pool(name="ps", bufs=1, space="PSUM"))

            xb = [cp.tile([128, W], F32, name=f"xb{b}", tag=f"xb{b}") for b in range(nb)]
            zb = cp.tile([DZ, W], F32, tag="zb")
            ax = cp.tile([2, W], F32, tag="ax")
            az = cp.tile([2, W], F32, tag="az")
            idf = cp.tile([128, 128], F32, tag="idf")
            id_sb = cp.tile([128, 128], BF16, tag="ident")
            s_sb = cp.tile([128, N], F32, tag="s_sb")    # stacked distances
            rr_sb = cp.tile([128, N], F32, tag="rr")

            for b in range(nb):
                nc.sync.dma_start(xb[b][0:64, :], xcat[b * 128:b * 128 + 64, :])
                nc.sync.dma_start(xb[b][64:128, :], xcat[b * 128 + 64:(b + 1) * 128, :])
            nc.sync.dma_start(zb[0:32, :], zcat[0:32, :])
            nc.sync.dma_start(zb[32:DZ, :], zcat[32:DZ, :])
            nc.sync.dma_start(ax[:], acx[:])
            nc.sync.dma_start(az[:], acz[:])
            nc.sync.dma_start(idf[:], ident[:])
            # identity -> bf16 via ScalarE so the k-loop matmuls depend only
            # on the ScalarE semaphore (one wait per matmul)
            nc.scalar.copy(id_sb[:], idf[:])

            g_s = pp.tile([128, N], F32, tag="g_s")
            s_ps = pp.tile([128, N], F32, tag="s_ps")
            r_ps = pp.tile([128, N], F32, tag="r_ps")

            # G' matmuls (contraction over features + 2 aux rows).
            # x-rows land on PSUM partitions 0-63, z-rows on 64-127 via
            # PE column tiling, so one ACT covers both distance slabs.
            for b in range(nb):
                nc.tensor.matmul(g_s[0:ROWS, :], xb[b][:, N:W], xb[b][:, 0:N],
                                 start=(b == 0), stop=False)
            nc.tensor.matmul(g_s[0:ROWS, :], ax[:, N:W], ax[:, 0:N],
                             start=False, stop=True)
            nc.tensor.matmul(g_s[ROWS:2 * ROWS, :], zb[:, N:W], zb[:, 0:N],
                             start=True, stop=False, tile_position=(0, ROWS))
            nc.tensor.matmul(g_s[ROWS:2 * ROWS, :], az[:, N:W], az[:, 0:N],
                             start=False, stop=True, tile_position=(0, ROWS))

            # distances: S = sqrt(max(-2 G', 0)) for both stacked slabs
            # (clamp on VectorE: keeps ScalarE to two ACT table sets)
            nc.vector.tensor_scalar(rr_sb[:], g_s[:], -2.0 / (TAU * TAU), 0.0,
                                    mybir.AluOpType.mult, mybir.AluOpType.max)
            nc.scalar.activation(s_sb[:], rr_sb[:], AF.Sqrt)
            # PSUM copy of S: ACT reads are cheaper from PSUM (172 vs 222 cyc)
            nc.vector.tensor_copy(s_ps[:], s_sb[:])

            # Dummy 1-elem sigmoid: forces the sigmoid ACT-table load here,
            # so the first real sigmoid doesn't pay an implicit table-load
            # (which costs it a sync-wait slot in walrus codegen).
            warm = cp.tile([1, 1], F32, tag="warm")
            nc.scalar.activation(warm[:], rr_sb[0:1, 0:1], AF.Sigmoid)

            # O(n^3) soft-rank, triangular: instruction k computes
            #   T_k[p, j] = sigmoid(S[p,k] - S[p,j])   for j < k only.
            # The j > k half follows from sigmoid(u) + sigmoid(-u) = 1:
            #   rank[p,j] = 1.5 + j + sum_{k>j} T_k[p,j] - sum_{k<j} T_j[p,k]
            # PE accumulates the first (cross-instruction) sum into r_ps;
            # VectorE row-reduces each T_k into w_sb[:, k] for the second.
            # The (1.5 + j) terms cancel in rank_z - rank_x on the host.
            w_sb = cp.tile([128, N], F32, tag="w_sb")
            nc.vector.memset(w_sb[:, 0:1], 0.0)
            # k descends so the first (start=True) matmul covers the widest
            # PSUM region; later ones write subsets of already-initialized
            # columns (PSUM lazy-zeroing is per start-region).
            for k in range(N - 1, 0, -1):
                tk = tkp.tile([128, k], BF16, name="tk", tag="tk")
                nc.scalar.activation(tk[:], s_ps[:, 0:k], AF.Sigmoid,
                                     bias=s_sb[:, k:k + 1], scale=-1.0)
                nc.vector.tensor_reduce(w_sb[:, k:k + 1], tk[:],
                                        axis=mybir.AxisListType.X,
                                        op=mybir.AluOpType.add)
                nc.tensor.matmul(r_ps[:, 0:k], id_sb[:], tk[:],
                                 start=(k == N - 1), stop=(k == 1))

            # V = R_acc - W ; host computes D = V[64:] - V[:64]
            r_sb = cp.tile([128, N], F32, tag="r_sb")
            # column N-1 has no k>j terms: V = 0 - W there (avoids touching
            # the PSUM accumulator bank from another engine mid-group)
            nc.vector.tensor_sub(r_sb[:, 0:N - 1], r_ps[:, 0:N - 1],
                                 w_sb[:, 0:N - 1])
            nc.vector.tensor_scalar_mul(r_sb[:, N - 1:N], w_sb[:, N - 1:N], -1.0)
            # SWDGE: static HWDGE DMAs have a single sync-wait slot, and this
            # one needs waits on both the DVE copy and the DMA queue clock.
            nc.gpsimd.dma_start(rout[:], r_sb[:])

    nc.compile()
    return nc


_CACHE: dict = {}


def _get_nc() -> bass.Bass:
    if "nc" not in _CACHE:
        _CACHE["nc"] = _build()
    return _CACHE["nc"]


def make_in_maps(x: np.ndarray, z: np.ndarray) -> list[dict]:
    x = np.ascontiguousarray(np.asarray(x, np.float32))
    z = np.ascontiguousarray(np.asarray(z, np.float32))
    sqx = (x * x).sum(1, dtype=np.float32)
    sqz = (z * z).sum(1, dtype=np.float32)
    xt = np.ascontiguousarray(x.T)
    zt = np.ascontiguousarray(z.T)
    axr = np.stack([np.ones(N, np.float32), sqx])
    azr = np.stack([np.ones(N, np.float32), sqz])
    ident = np.eye(128, dtype=np.float32)
    in_maps = []
    for c in range(NCORES):
        s = slice(c * ROWS, (c + 1) * ROWS)
        axl = np.stack([-sqx[s] / 2, np.full(ROWS, -0.5, np.float32)])
        azl = np.stack([-sqz[s] / 2, np.full(ROWS, -0.5, np.float32)])
        in_maps.append({
            "xcat": np.ascontiguousarray(np.concatenate([xt, xt[:, s]], 1)),
            "zcat": np.ascontiguousarray(np.concatenate([zt, zt[:, s]], 1)),
            "acx": np.ascontiguousarray(np.concatenate([axr, axl], 1)),
            "acz": np.ascontiguousarray(np.concatenate([azr, azl], 1)),
            "ident": ident,
        })
    return in_maps


def finish(routs: list[np.ndarray]):
    ss = 0.0
    for c in range(NCORES):
        R = np.asarray(routs[c], np.float64)
        D = R[ROWS:2 * ROWS] - R[:ROWS]
        ss += (D * D).sum()
    loss = np.float32(ss / (N * N))
    return (loss, loss, np.float32(0.0))


def kernel(x: np.ndarray, z: np.ndarray):
    nc = _get_nc()
    in_maps = make_in_maps(x, z)
    res = run_bass_kernel_spmd(nc, in_maps, list(range(NCORES)))
    _CACHE["last_result"] = res
    return finish([res.results[c]["rout"] for c in range(NCORES)])

